# revision 13
# baseline (speedup 1.0000x reference)
"""
DistanceSampling Trainium2 kernel (8 NeuronCores, SPMD over patch rows).

Computation per 2x2/stride-2 patch of x (1, 256, 512, 512) fp32:
  mean over the 4 patch elements (per channel), d_k = ||x_k - mean + eps||_2
  over channels, k* = argmax_k d_k (first occurrence), out = x_{k*}.
Output: (1, 256, 65536) fp32.

Sharding: core m gets image rows [64m, 64m+64) = 32 patch rows = 8192 patch
locations; fully independent, no collectives. Output chunks concatenated on
the host along L.

Per-core design (16 qpairs of 2 patch rows x 256 cols = 512 locations):
  channels on SBUF partitions (2 blocks of 128), locations on the free dim.

  Distance differences via a sum/difference-of-squares identity: with
  a = x0+x1, b = x2+x3, A0 = 2*x0-b, A1 = 2*x1-b, B2 = 2*x2-a, B3 = 2*x3-a,
  the six pairwise distance differences (x16, eps dropped - measured 0
  argmax flips) are exact +-{1,2,3} linear combinations of the channel
  sums of A0^2, A1^2, B2^2, B3^2. So per channel-block only three
  elementwise ops (one pair-sum + two fused scale-subtract ops) and one
  Square feed eight accumulating fp32 matmuls that emit the 6 diffs
  directly into PSUM.

  Argmax masks: u = (diff > 0), beats-count matmul (+-1), is_equal vs
  [0,1,2,3] -> exact first-occurrence one-hot. Selection via GpSimd
  ap_gather: two tiny matmuls turn the one-hot into per-location gather
  offsets into the X tile (+ base column offset), converted to int16 and
  wrap-transposed by a small SBUF DMA into the [128, 32] interleaved
  index layout ap_gather expects; one gather per channel block replaces
  all mask broadcasts and predicated copies.

  Locations are enumerated in a permuted column order lam(c) =
  16*(c%32) + c//32 end to end, which makes the index wrap-DMA and the
  output DMA both contiguous (ap_gather's fixed interleaved unwrap then
  restores the natural order).

All arithmetic fp32 (exact +-1/2/3 and small-integer fp16 constants
elsewhere), so argmax decisions match the reference up to fp32 rounding
order; measured 0 flipped locations on the reference input (host emu).
"""

import sys

sys.path.insert(0, "/opt/trn_rl_repo")

import numpy as np

import concourse.bacc as bacc
import concourse.bass as bass
import concourse.mybir as mybir
import concourse.tile as tile
from concourse.bass_utils import run_bass_kernel_spmd

f32 = mybir.dt.float32
f16 = mybir.dt.float16
bf16 = mybir.dt.bfloat16
i16 = mybir.dt.int16
Alu = mybir.AluOpType
Act = mybir.ActivationFunctionType

C, H, W = 256, 512, 512
NCORES = 8
RPC = H // NCORES  # image rows per core (64)
QP = 16  # qpair groups per core (4 image rows each)
LPC = 8192  # locations per core


def _kernel_body(tc):
    nc = tc.nc
    x = nc.dram_tensor("x", [C, RPC, W], f32, kind="ExternalInput").ap()
    cW = nc.dram_tensor("cW", [128, 24], f32, kind="ExternalInput").ap()
    cM = nc.dram_tensor("cM", [6, 4], bf16, kind="ExternalInput").ap()
    cneed = nc.dram_tensor("cneed", [4, 1], f32, kind="ExternalInput").ap()
    cOFF = nc.dram_tensor("cOFF", [4, 8], f16, kind="ExternalInput").ap()
    cONE = nc.dram_tensor("cONE", [1, 8], f16, kind="ExternalInput").ap()
    cBASE = nc.dram_tensor("cBASE", [1, 512], f16, kind="ExternalInput").ap()
    out = nc.dram_tensor("out", [C, LPC], f32, kind="ExternalOutput").ap()

    with (
        tc.tile_pool(name="const", bufs=1) as constp,
        tc.tile_pool(name="xin", bufs=5) as xp,
        tc.tile_pool(name="stile", bufs=2) as stp,
        tc.tile_pool(name="ab", bufs=2) as abp,
        tc.tile_pool(name="sq", bufs=2) as sqp,
        tc.tile_pool(name="small", bufs=4) as smp,
        tc.tile_pool(name="idx", bufs=4) as ixp,
        tc.tile_pool(name="ot", bufs=2) as otp,
        tc.tile_pool(name="ps_diff", bufs=2, space=bass.MemorySpace.PSUM) as pd,
        tc.tile_pool(name="ps_b", bufs=2, space=bass.MemorySpace.PSUM) as pb,
        tc.tile_pool(name="ps_idx", bufs=2, space=bass.MemorySpace.PSUM) as pi,
    ):
        W_t = constp.tile([128, 24], f32)
        nc.sync.dma_start(W_t[:], cW)
        M_t = constp.tile([6, 4], bf16)
        nc.sync.dma_start(M_t[:], cM)
        need_t = constp.tile([4, 1], f32)
        nc.sync.dma_start(need_t[:], cneed)
        OFF_t = constp.tile([4, 8], f16)
        nc.sync.dma_start(OFF_t[:], cOFF)
        ONE_t = constp.tile([1, 8], f16)
        nc.sync.dma_start(ONE_t[:], cONE)
        BASE_t = constp.tile([1, 512], f16)
        nc.sync.dma_start(BASE_t[:], cBASE)

        def stage_load(qp):
            Xs = []
            for cb in range(2):
                X = xp.tile([128, 2048], f32, tag=f"X{cb}")
                nc.sync.dma_start(
                    X[:], x[cb * 128 : (cb + 1) * 128, 4 * qp : 4 * qp + 4, :]
                )
                Xs.append(X)
            return Xs

        def stage_prep(qp, Xs):
            Ss = []
            for cb in range(2):
                X = Xs[cb]
                # pair sums st[p, a*512 + h*256 + f]: contiguous stride-2 APs
                xe = X[:].rearrange("p (q s) -> p q s", s=2)
                st = stp.tile([128, 1024], f32, tag=f"s{cb}")
                nc.vector.tensor_tensor(st[:], xe[:, :, 0], xe[:, :, 1], Alu.add)
                stv = st[:].rearrange("p (a h f) -> p h a f", a=2, h=2)
                # sha = (x0+x1)/2 per location (for the GpSimd B ops)
                sha = stp.tile([128, 512], f32, tag=f"sh{cb}")
                nc.scalar.activation(
                    sha[:].rearrange("p (a f) -> p a f", a=2),
                    stv[:, 0], Act.Copy, scale=0.5,
                )
                # AB = [A0|A1|B2'|B3']: A_k = 2*x_k - b (stt, DVE);
                # B'_k = x_k - a/2 = B_k/2 (tensor_tensor, GpSimd); the /2
                # is compensated exactly in the matmul coefficients (x4).
                AB = abp.tile([128, 2048], f32, tag=f"D{cb}")
                xk4 = X[:].rearrange("p (a h f s) -> p h s a f", a=2, h=2, s=2)
                for k, (hk, sk) in enumerate(((0, 0), (0, 1), (1, 0), (1, 1))):
                    ov = AB[:, k * 512 : (k + 1) * 512].rearrange(
                        "p (a f) -> p a f", a=2
                    )
                    if k < 2:
                        nc.vector.scalar_tensor_tensor(
                            ov, xk4[:, 0, sk], 2.0, stv[:, 1],
                            Alu.mult, Alu.subtract,
                        )
                    else:
                        nc.gpsimd.tensor_tensor(
                            ov, xk4[:, 1, sk],
                            sha[:].rearrange("p (a f) -> p a f", a=2),
                            Alu.subtract,
                        )
                S = sqp.tile([128, 2048], f32, tag=f"S{cb}")
                nc.scalar.activation(S[:], AB[:], Act.Square)
                Ss.append(S)
            dps = pd.tile([6, 512], f32, tag="diff")
            for cb in range(2):
                for t in range(4):
                    nc.tensor.matmul(
                        dps[:],
                        W_t[:, 6 * t : 6 * t + 6],
                        Ss[cb][:, 512 * t : 512 * (t + 1)],
                        start=(cb == 0 and t == 0),
                        stop=(cb == 1 and t == 3),
                    )
            return Xs, dps

        def stage_mask(dps):
            # u = 1{diff > 0} as relu(sign(diff)), both on Act
            sg = smp.tile([6, 512], bf16, tag="sg")
            nc.scalar.activation(sg[:], dps[:], Act.Sign)
            u = smp.tile([6, 512], bf16, tag="u")
            nc.scalar.activation(u[:], sg[:], Act.Relu)
            bps = pb.tile([4, 512], f32, tag="b")
            nc.tensor.matmul(bps[:], M_t[:], u[:], start=True, stop=True)
            m = smp.tile([4, 512], f16, tag="m")
            nc.vector.tensor_scalar(
                out=m[:], in0=bps[:], scalar1=need_t[:], scalar2=None,
                op0=Alu.is_equal,
            )
            # gather index = one-hot . OFF + BASE; the OFF matmul reads m's
            # columns in lam order (3-dim rhs AP) so the idx row is stored
            # wrap-transposable; BASE is host-permuted to match.
            ips = pi.tile([8, 512], f32, tag="idx")
            mlam = m[:].rearrange("p (s w) -> p w s", s=32, w=16)
            nc.tensor.matmul(ips[:], OFF_t[:], mlam, start=True, stop=False)
            nc.tensor.matmul(ips[:], ONE_t[:], BASE_t[:], start=False, stop=True)
            idx16 = ixp.tile([8, 512], i16, tag="i16")
            nc.scalar.activation(idx16[:], ips[:], Act.Copy)
            idxw = ixp.tile([128, 32], i16, tag="iw")
            # issue on the Act engine's DGE ring so this tiny transfer is
            # not FIFO-ordered behind the bulk X loads on the SP ring
            nc.scalar.dma_start(
                idxw[:], idx16[:].rearrange("p (w s) -> p w s", w=16, s=32)
            )
            return idxw

        def stage_gather(qp, Xs, idxw):
            for cb in range(2):
                ot = otp.tile([128, 512], f32, tag=f"o{cb}")
                nc.gpsimd.ap_gather(
                    ot[:], Xs[cb][:], idxw[:],
                    channels=128, num_elems=2048, d=1, num_idxs=512,
                )
                # Act DGE ring: keep the SP ring free-running with X loads
                nc.scalar.dma_start(
                    out[cb * 128 : (cb + 1) * 128, qp * 512 : (qp + 1) * 512],
                    ot[:],
                )

        # two-stage software pipeline skew: emit mask(i-1) before prep(i)
        # and gather(i-2) after, so the in-order engine queues never stall
        # on the long diff->mask->index dependency chain.
        state = {}
        loads = {}
        for i in range(QP + 2):
            if i < QP:
                loads[i] = stage_load(i)
            if 1 <= i <= QP:
                qm = i - 1
                state[qm] = (state[qm][0], stage_mask(state[qm][1]))
            if i < QP:
                state[i] = stage_prep(i, loads.pop(i))
            if i >= 2:
                qg = i - 2
                Xs, idxw = state[qg]
                stage_gather(qg, Xs, idxw)
                del state[qg]

def _const_arrays():
    import ml_dtypes

    # Delta_j = d_a - d_b (pair order (1,0),(2,0),(2,1),(3,0),(3,1),(3,2))
    # as exact linear combos of the channel sums of (A0^2, A1^2, B2^2, B3^2)
    coeffs = [
        (-2, 2, 0, 0),
        (-3, -1, 12, 4),
        (-1, -3, 12, 4),
        (-3, -1, 4, 12),
        (-1, -3, 4, 12),
        (0, 0, -8, 8),
    ]
    Warr = np.zeros((128, 24), np.float32)
    for j, cf in enumerate(coeffs):
        for t in range(4):
            Warr[:, 6 * t + j] = cf[t]
    M = np.array(
        [
            [-1, 1, 0, 0],
            [-1, 0, 1, 0],
            [0, -1, 1, 0],
            [-1, 0, 0, 1],
            [0, -1, 0, 1],
            [0, 0, -1, 1],
        ],
        np.float32,
    ).astype(ml_dtypes.bfloat16)
    need = np.array([[0.0], [1.0], [2.0], [3.0]], np.float32)
    OFF = np.zeros((4, 8), np.float32)
    for k, off in enumerate((0.0, 1.0, 512.0, 513.0)):
        OFF[k, :] = off
    ONE = np.ones((1, 8), np.float32)
    # BASE[c] = a*1024 + 2*f of location lam(c) = 16*(c%32) + c//32
    cpos = np.arange(512)
    lam = 16 * (cpos % 32) + cpos // 32
    BASE = ((lam // 256) * 1024 + 2 * (lam % 256)).astype(np.float32)[None]
    return {
        "cW": Warr,
        "cM": M,
        "cneed": need,
        "cOFF": OFF.astype(np.float16),
        "cONE": ONE.astype(np.float16),
        "cBASE": BASE.astype(np.float16),
    }


_compiled_nc = None


def _get_compiled():
    global _compiled_nc
    if _compiled_nc is None:
        nc = bacc.Bacc(
            "TRN2", target_bir_lowering=False, debug=False, num_devices=NCORES
        )
        with tile.TileContext(nc) as tc:
            _kernel_body(tc)
        nc.compile()
        _compiled_nc = nc
    return _compiled_nc


def run_sharded(x_full: np.ndarray, **spmd_kwargs):
    """x_full: (1, C, H, W) fp32. Returns (results, raw) where results is the
    assembled (1, C, L) array and raw is the BassKernelResults."""
    nc = _get_compiled()
    xs = x_full[0]  # (C, H, W)
    consts = _const_arrays()
    in_maps = [
        {"x": np.ascontiguousarray(xs[:, m * RPC : (m + 1) * RPC, :]), **consts}
        for m in range(NCORES)
    ]
    raw = run_bass_kernel_spmd(nc, in_maps, list(range(NCORES)), **spmd_kwargs)
    outs = [raw.results[m]["out"] for m in range(NCORES)]  # (C, LPC) each
    full = np.concatenate(outs, axis=1)[None]  # (1, C, L)
    return full, raw


def kernel(x: np.ndarray) -> np.ndarray:
    x = np.asarray(x, dtype=np.float32)
    assert x.shape == (1, C, H, W), x.shape
    full, _ = run_sharded(x)
    return full


# revision 14
# speedup vs baseline: 4.2077x; 4.2077x over previous
"""
DistanceSampling Trainium2 kernel (8 NeuronCores, SPMD over patch rows).

Computation per 2x2/stride-2 patch of x (1, 256, 512, 512) fp32:
  mean over the 4 patch elements (per channel), d_k = ||x_k - mean + eps||_2
  over channels, k* = argmax_k d_k (first occurrence), out = x_{k*}.
Output: (1, 256, 65536) fp32.

Sharding: core m gets image rows [64m, 64m+64) = 32 patch rows = 8192 patch
locations; fully independent, no collectives. Output chunks concatenated on
the host along L.

Per-core design (16 qpairs of 2 patch rows x 256 cols = 512 locations):
  channels on SBUF partitions (2 blocks of 128), locations on the free dim.

  Distance differences via a sum/difference-of-squares identity: with
  a = x0+x1, b = x2+x3, A0 = 2*x0-b, A1 = 2*x1-b, B2 = 2*x2-a, B3 = 2*x3-a,
  the six pairwise distance differences (x16, eps dropped - measured 0
  argmax flips) are exact +-{1,2,3} linear combinations of the channel
  sums of A0^2, A1^2, B2^2, B3^2. So per channel-block only three
  elementwise ops (one pair-sum + two fused scale-subtract ops) and one
  Square feed eight accumulating fp32 matmuls that emit the 6 diffs
  directly into PSUM.

  Argmax masks: u = (diff > 0), beats-count matmul (+-1), is_equal vs
  [0,1,2,3] -> exact first-occurrence one-hot. Selection via GpSimd
  ap_gather: two tiny matmuls turn the one-hot into per-location gather
  offsets into the X tile (+ base column offset), converted to int16 and
  wrap-transposed by a small SBUF DMA into the [128, 32] interleaved
  index layout ap_gather expects; one gather per channel block replaces
  all mask broadcasts and predicated copies.

  Locations are enumerated in a permuted column order lam(c) =
  16*(c%32) + c//32 end to end, which makes the index wrap-DMA and the
  output DMA both contiguous (ap_gather's fixed interleaved unwrap then
  restores the natural order).

All arithmetic fp32 (exact +-1/2/3 and small-integer fp16 constants
elsewhere), so argmax decisions match the reference up to fp32 rounding
order; measured 0 flipped locations on the reference input (host emu).
"""

import sys

sys.path.insert(0, "/opt/trn_rl_repo")

import numpy as np

import concourse.bacc as bacc
import concourse.bass as bass
import concourse.mybir as mybir
import concourse.tile as tile
from concourse.bass_utils import run_bass_kernel_spmd

f32 = mybir.dt.float32
f16 = mybir.dt.float16
bf16 = mybir.dt.bfloat16
i16 = mybir.dt.int16
Alu = mybir.AluOpType
Act = mybir.ActivationFunctionType

C, H, W = 256, 512, 512
NCORES = 8
RPC = H // NCORES  # image rows per core (64)
QP = 16  # qpair groups per core (4 image rows each)
LPC = 8192  # locations per core


def _kernel_body(tc):
    nc = tc.nc
    x = nc.dram_tensor("x", [C, RPC, W], f32, kind="ExternalInput").ap()
    cW = nc.dram_tensor("cW", [128, 24], f32, kind="ExternalInput").ap()
    cM = nc.dram_tensor("cM", [6, 4], bf16, kind="ExternalInput").ap()
    cneed = nc.dram_tensor("cneed", [4, 1], f32, kind="ExternalInput").ap()
    cOFF = nc.dram_tensor("cOFF", [4, 8], f16, kind="ExternalInput").ap()
    cONE = nc.dram_tensor("cONE", [1, 8], f16, kind="ExternalInput").ap()
    cBASE = nc.dram_tensor("cBASE", [1, 512], f16, kind="ExternalInput").ap()
    out = nc.dram_tensor("out", [C, LPC], f32, kind="ExternalOutput").ap()

    with (
        tc.tile_pool(name="const", bufs=1) as constp,
        tc.tile_pool(name="xin", bufs=5) as xp,
        tc.tile_pool(name="stile", bufs=2) as stp,
        tc.tile_pool(name="ab", bufs=2) as abp,
        tc.tile_pool(name="sq", bufs=2) as sqp,
        tc.tile_pool(name="small", bufs=3) as smp,
        tc.tile_pool(name="idx", bufs=3) as ixp,
        tc.tile_pool(name="ot", bufs=2) as otp,
        tc.tile_pool(name="ps_diff", bufs=3, space=bass.MemorySpace.PSUM) as pd,
        tc.tile_pool(name="ps_b", bufs=2, space=bass.MemorySpace.PSUM) as pb,
        tc.tile_pool(name="ps_idx", bufs=2, space=bass.MemorySpace.PSUM) as pi,
    ):
        W_t = constp.tile([128, 24], f32)
        nc.sync.dma_start(W_t[:], cW)
        M_t = constp.tile([6, 4], bf16)
        nc.sync.dma_start(M_t[:], cM)
        need_t = constp.tile([4, 1], f32)
        nc.sync.dma_start(need_t[:], cneed)
        OFF_t = constp.tile([4, 8], f16)
        nc.sync.dma_start(OFF_t[:], cOFF)
        ONE_t = constp.tile([1, 8], f16)
        nc.sync.dma_start(ONE_t[:], cONE)
        BASE_t = constp.tile([1, 512], f16)
        nc.sync.dma_start(BASE_t[:], cBASE)

        def stage_load(qp):
            Xs = []
            for cb in range(2):
                X = xp.tile([128, 2048], f32, tag=f"X{cb}")
                nc.sync.dma_start(
                    X[:], x[cb * 128 : (cb + 1) * 128, 4 * qp : 4 * qp + 4, :]
                )
                Xs.append(X)
            return Xs

        def stage_prep(qp, Xs):
            Ss = []
            for cb in range(2):
                X = Xs[cb]
                # pair sums st[p, a*512 + h*256 + f]: contiguous stride-2 APs
                xe = X[:].rearrange("p (q s) -> p q s", s=2)
                st = stp.tile([128, 1024], f32, tag=f"s{cb}")
                nc.vector.tensor_tensor(st[:], xe[:, :, 0], xe[:, :, 1], Alu.add)
                stv = st[:].rearrange("p (a h f) -> p h a f", a=2, h=2)
                # sha = (x0+x1)/2 per location (for the GpSimd B ops)
                sha = stp.tile([128, 512], f32, tag=f"sh{cb}")
                nc.scalar.activation(
                    sha[:].rearrange("p (a f) -> p a f", a=2),
                    stv[:, 0], Act.Copy, scale=0.5,
                )
                # AB = [A0|A1|B2'|B3']: A_k = 2*x_k - b (stt, DVE);
                # B'_k = x_k - a/2 = B_k/2 (tensor_tensor, GpSimd); the /2
                # is compensated exactly in the matmul coefficients (x4).
                AB = abp.tile([128, 2048], f32, tag=f"D{cb}")
                xk4 = X[:].rearrange("p (a h f s) -> p h s a f", a=2, h=2, s=2)
                for k, (hk, sk) in enumerate(((0, 0), (0, 1), (1, 0), (1, 1))):
                    ov = AB[:, k * 512 : (k + 1) * 512].rearrange(
                        "p (a f) -> p a f", a=2
                    )
                    if k < 2:
                        nc.vector.scalar_tensor_tensor(
                            ov, xk4[:, 0, sk], 2.0, stv[:, 1],
                            Alu.mult, Alu.subtract,
                        )
                    else:
                        nc.gpsimd.tensor_tensor(
                            ov, xk4[:, 1, sk],
                            sha[:].rearrange("p (a f) -> p a f", a=2),
                            Alu.subtract,
                        )
                S = sqp.tile([128, 2048], f32, tag=f"S{cb}")
                nc.scalar.activation(S[:], AB[:], Act.Square)
                Ss.append(S)
            dps = pd.tile([6, 512], f32, tag="diff")
            for cb in range(2):
                for t in range(4):
                    nc.tensor.matmul(
                        dps[:],
                        W_t[:, 6 * t : 6 * t + 6],
                        Ss[cb][:, 512 * t : 512 * (t + 1)],
                        start=(cb == 0 and t == 0),
                        stop=(cb == 1 and t == 3),
                    )
            return Xs, dps

        def stage_isgt(dps):
            u = smp.tile([6, 512], bf16, tag="u")
            nc.vector.tensor_scalar(
                out=u[:], in0=dps[:], scalar1=0.0, scalar2=None, op0=Alu.is_gt
            )
            return u

        def stage_beats(u):
            bps = pb.tile([4, 512], f32, tag="b")
            nc.tensor.matmul(bps[:], M_t[:], u[:], start=True, stop=True)
            return bps

        def stage_iseq(bps):
            m = smp.tile([4, 512], f16, tag="m")
            nc.vector.tensor_scalar(
                out=m[:], in0=bps[:], scalar1=need_t[:], scalar2=None,
                op0=Alu.is_equal,
            )
            return m

        def stage_off(m):
            # gather index = one-hot . OFF + BASE; the OFF matmul reads m's
            # columns in lam order (3-dim rhs AP) so the idx row is stored
            # wrap-transposable; BASE is host-permuted to match.
            ips = pi.tile([8, 512], f32, tag="idx")
            mlam = m[:].rearrange("p (s w) -> p w s", s=32, w=16)
            nc.tensor.matmul(ips[:], OFF_t[:], mlam, start=True, stop=False)
            nc.tensor.matmul(ips[:], ONE_t[:], BASE_t[:], start=False, stop=True)
            return ips

        def stage_cvt(ips):
            idx16 = ixp.tile([8, 512], i16, tag="i16")
            nc.vector.tensor_scalar(
                out=idx16[:], in0=ips[:], scalar1=0.0, scalar2=None, op0=Alu.add
            )
            idxw = ixp.tile([128, 32], i16, tag="iw")
            # Act engine DGE ring: not FIFO-ordered behind the X loads
            nc.scalar.dma_start(
                idxw[:], idx16[:].rearrange("p (w s) -> p w s", w=16, s=32)
            )
            return idxw

        def stage_gather(qp, Xs, idxw):
            for cb in range(2):
                ot = otp.tile([128, 512], f32, tag=f"o{cb}")
                nc.gpsimd.ap_gather(
                    ot[:], Xs[cb][:], idxw[:],
                    channels=128, num_elems=2048, d=1, num_idxs=512,
                )
                nc.scalar.dma_start(
                    out[cb * 128 : (cb + 1) * 128, qp * 512 : (qp + 1) * 512],
                    ot[:],
                )

        # Deep skewed pipeline: every op an engine dequeues is already
        # ready, so the in-order engine queues never stall on the long
        # diff -> mask -> index -> gather chain.
        #   iter i: load(i); is_gt(i-1); is_eq(i-2); prep(i) [s/sha/AB/SQ/
        #   S-matmuls]; M(i-1); OFF+BASE(i-2); gather(i-3); cvt+wrap(i-2)
        st_ = {}
        for i in range(QP + 3):
            if i < QP:
                st_[i] = {"Xs": stage_load(i)}
            if 1 <= i <= QP:
                st_[i - 1]["u"] = stage_isgt(st_[i - 1]["dps"])
            if 2 <= i <= QP + 1:
                st_[i - 2]["m"] = stage_iseq(st_[i - 2]["bps"])
            if i < QP:
                _, st_[i]["dps"] = stage_prep(i, st_[i]["Xs"])
            if 1 <= i <= QP:
                st_[i - 1]["bps"] = stage_beats(st_[i - 1]["u"])
            if 2 <= i <= QP + 1:
                st_[i - 2]["ips"] = stage_off(st_[i - 2]["m"])
            if 3 <= i <= QP + 2:
                qg = i - 3
                stage_gather(qg, st_[qg]["Xs"], st_[qg]["idxw"])
            if 2 <= i <= QP + 1:
                st_[i - 2]["idxw"] = stage_cvt(st_[i - 2]["ips"])

def _const_arrays():
    import ml_dtypes

    # Delta_j = d_a - d_b (pair order (1,0),(2,0),(2,1),(3,0),(3,1),(3,2))
    # as exact linear combos of the channel sums of (A0^2, A1^2, B2^2, B3^2)
    coeffs = [
        (-2, 2, 0, 0),
        (-3, -1, 12, 4),
        (-1, -3, 12, 4),
        (-3, -1, 4, 12),
        (-1, -3, 4, 12),
        (0, 0, -8, 8),
    ]
    Warr = np.zeros((128, 24), np.float32)
    for j, cf in enumerate(coeffs):
        for t in range(4):
            Warr[:, 6 * t + j] = cf[t]
    M = np.array(
        [
            [-1, 1, 0, 0],
            [-1, 0, 1, 0],
            [0, -1, 1, 0],
            [-1, 0, 0, 1],
            [0, -1, 0, 1],
            [0, 0, -1, 1],
        ],
        np.float32,
    ).astype(ml_dtypes.bfloat16)
    need = np.array([[0.0], [1.0], [2.0], [3.0]], np.float32)
    OFF = np.zeros((4, 8), np.float32)
    for k, off in enumerate((0.0, 1.0, 512.0, 513.0)):
        OFF[k, :] = off
    ONE = np.ones((1, 8), np.float32)
    # BASE[c] = a*1024 + 2*f of location lam(c) = 16*(c%32) + c//32
    cpos = np.arange(512)
    lam = 16 * (cpos % 32) + cpos // 32
    BASE = ((lam // 256) * 1024 + 2 * (lam % 256)).astype(np.float32)[None]
    return {
        "cW": Warr,
        "cM": M,
        "cneed": need,
        "cOFF": OFF.astype(np.float16),
        "cONE": ONE.astype(np.float16),
        "cBASE": BASE.astype(np.float16),
    }


_compiled_nc = None


def _get_compiled():
    global _compiled_nc
    if _compiled_nc is None:
        nc = bacc.Bacc(
            "TRN2", target_bir_lowering=False, debug=False, num_devices=NCORES
        )
        with tile.TileContext(nc) as tc:
            _kernel_body(tc)
        nc.compile()
        _compiled_nc = nc
    return _compiled_nc


def run_sharded(x_full: np.ndarray, **spmd_kwargs):
    """x_full: (1, C, H, W) fp32. Returns (results, raw) where results is the
    assembled (1, C, L) array and raw is the BassKernelResults."""
    nc = _get_compiled()
    xs = x_full[0]  # (C, H, W)
    consts = _const_arrays()
    in_maps = [
        {"x": np.ascontiguousarray(xs[:, m * RPC : (m + 1) * RPC, :]), **consts}
        for m in range(NCORES)
    ]
    raw = run_bass_kernel_spmd(nc, in_maps, list(range(NCORES)), **spmd_kwargs)
    outs = [raw.results[m]["out"] for m in range(NCORES)]  # (C, LPC) each
    full = np.concatenate(outs, axis=1)[None]  # (1, C, L)
    return full, raw


def kernel(x: np.ndarray) -> np.ndarray:
    x = np.asarray(x, dtype=np.float32)
    assert x.shape == (1, C, H, W), x.shape
    full, _ = run_sharded(x)
    return full
